# revision 34
# baseline (speedup 1.0000x reference)
"""BrickTube kernel for 8x Trainium2 NeuronCores.

The reference "BrickTube" module applies 80 tiny (2,2,2,2) gate cores to a
[B, 1024] state tensor. Every gate application is linear in x and
INPUT_DIM == BINDIM == OUTPUT_DIM == 1024, so the whole module collapses to

    out = x @ W,   W[i, :] = circuit(e_i)  (1024 x 1024)

W is built exactly on the host in float64 from `cores` (cheap: 80 small
tensordots), then the device runs a batch-sharded dense fp16 matmul with
fp32 PSUM accumulation: each of the 8 cores computes y_c^T = W^T @ x_c^T
for its 4096-row shard of x. (fp8 DoubleRow was evaluated and discarded:
the per-matmul LDWEIGHTS issue rate (~109ns per 128-col stationary swap)
caps the DoubleRow stream at fp16-equivalent throughput for this shape.)

Schedule notes (per core, j-outer over 8 batch chunks of 512 columns):
  - j0..j6: k-outer/m-inner into 8 PSUM banks; drains (DVE/ACT split by m
    parity) and pair output DMAs overlap the next chunk's matmuls. Output
    DMA triggers alternate the Sync/Scalar HWDGE rings.
  - j7: m-outer/k-inner so each bank finishes, drains, and stores while
    the next bank computes; the final bank is drained and stored as two
    64KB halves on both rings, so the kernel tail is minimal.
  - Output is fp16 (rounding adds ~2e-4 relative error; the gate is 2e-2),
    halving output traffic and drain cost vs fp32.
  - ~3us of warmup matmuls on zeros while the first DMAs land bring the
    PE clock (HAM ramp) to full rate before real data arrives. The first
    x piece is a single 128KB k-chunk and the first w piece is a 32KB
    column sliver so the first real matmul can start as early as possible.
"""

import numpy as np

# ---- problem constants (hardcoded per contract) ----
B = 32768
D = 1024
N_CORES = 8
NPC = B // N_CORES  # 4096 batch rows per core

BOND = 2
Q = 10
N_LAYERS = 8
PAIRS1 = [(i, i + 1) for i in range(0, Q, 2)]
PAIRS2 = [(i, (i + 1) % Q) for i in range(1, Q, 2)]
HALF = Q // 2

KC = D // 128      # 8 contraction chunks
JC = NPC // 512    # 8 batch column chunks
MC = D // 128      # 8 output-row chunks


def build_w(cores: np.ndarray) -> np.ndarray:
    """Collapse the 80-gate circuit into W [1024, 1024] (float64),
    with out_row = x_row @ W."""
    c = cores.astype(np.float64)
    s = np.eye(D, dtype=np.float64).reshape((D,) + (BOND,) * Q)
    for layer in range(N_LAYERS):
        base = layer * Q
        for g, (i, j) in enumerate(PAIRS1):
            s = np.tensordot(s, c[base + g], axes=((i + 1, j + 1), (0, 1)))
            s = np.moveaxis(s, (-2, -1), (i + 1, j + 1))
        for g, (i, j) in enumerate(PAIRS2):
            s = np.tensordot(s, c[base + HALF + g], axes=((i + 1, j + 1), (0, 1)))
            s = np.moveaxis(s, (-2, -1), (i + 1, j + 1))
    return s.reshape(D, D)


_NC_CACHE = None


def _build_bass():
    """Device program (identical on all 8 cores):
      inputs:  xt16 [D, NPC] f16            xt16[k, n] = x[n, k] (x-shard^T)
               w16  [128, KC, D] f16        (p,kc,m)   = W[kc*128+p, m]
      output:  yt16 [128, MC, NPC] f16      (p,m,n)    = y[n, m*128+p]
    """
    global _NC_CACHE
    if _NC_CACHE is not None:
        return _NC_CACHE

    import concourse.bacc as bacc
    import concourse.mybir as mybir
    import concourse.tile as tile

    F16 = mybir.dt.float16
    F32 = mybir.dt.float32

    nc = bacc.Bacc("TRN2")
    xt16 = nc.dram_tensor("xt16", [D, NPC], F16, kind="ExternalInput")
    w16 = nc.dram_tensor("w16", [128, KC, D], F16, kind="ExternalInput")
    yt16 = nc.dram_tensor("yt16", [128, MC, NPC], F16, kind="ExternalOutput")

    with tile.TileContext(nc) as tc:
        with (
            tc.tile_pool(name="xpool", bufs=1) as xpool,
            tc.tile_pool(name="wpool", bufs=1) as wpool,
            tc.tile_pool(name="opool", bufs=2) as opool,
            tc.tile_pool(name="psum", bufs=1, space="PSUM") as ppool,
        ):
            # ---- PE warmup (HAM clock ramp) while the first DMAs land.
            warm = xpool.tile([128, 64], F16, name="warm", tag="warm")
            nc.gpsimd.memset(warm[:], 0)
            wps = ppool.tile([128, 64], F32, name="wps", tag="ps7")
            for _ in range(56):
                nc.tensor.matmul(wps[0:64, :], warm[:], warm[:])

            # ---- input loads.
            # Sync ring: x. j0 per k-chunk (first rhs lands ASAP), j1 in two
            # halves, j2+ whole (8KB contiguous per partition row).
            x16t = []
            for j in range(JC):
                xtile = xpool.tile(
                    [128, KC * 512], F16, name=f"x16_{j}", tag=f"x16_{j}"
                )
                # j0 in 4 pieces of 2 k-chunks, j1 in 2 pieces, rest whole:
                # this piece cadence matches the DMA ring's ramp-up so the
                # matmul stream never starves once it starts (starting
                # earlier on finer pieces just stalls the stream and drops
                # the PE clock out of its ramped state).
                src = xt16[:, j * 512 : (j + 1) * 512]
                pieces = 4 if j == 0 else (2 if j == 1 else 1)
                kk = KC // pieces
                for p in range(pieces):
                    nc.sync.dma_start(
                        xtile[
                            :, p * kk * 512 : (p + 1) * kk * 512
                        ].rearrange("p (k n) -> p k n", n=512),
                        src[p * kk * 128 : (p + 1) * kk * 128, :].rearrange(
                            "(k p) n -> p k n", p=128
                        ),
                    )
                x16t.append(xtile)
            # Scalar ring: w. k0 in three pieces (first matmul only needs the
            # leading 128 columns), then k1..k7 whole.
            w16t = []
            for kc in range(KC):
                wt = wpool.tile([128, D], F16, name=f"w16_{kc}", tag=f"w16_{kc}")
                nc.scalar.dma_start(wt[:], w16[:, kc])
                w16t.append(wt)

            ndma = [0]

            def out_dma(dst, src, j):
                # mid-run outputs ride the Scalar ring (idle after the w
                # loads) so they never contend with x delivery on Sync;
                # late outputs (x long since delivered) alternate rings.
                if j < 6:
                    eng = nc.scalar
                else:
                    eng = nc.sync if ndma[0] % 2 == 0 else nc.scalar
                    ndma[0] += 1
                eng.dma_start(dst, src)

            def mm(ps, mc, kc, j, start, stop):
                nc.tensor.matmul(
                    ps[:],
                    w16t[kc][:, mc * 128 : (mc + 1) * 128],
                    x16t[j][:, kc * 512 : (kc + 1) * 512],
                    start=start,
                    stop=stop,
                )

            # ---- main loop.
            for j in range(JC):
                ps = [
                    ppool.tile([128, 512], F32, name=f"ps{m}", tag=f"ps{m}")
                    for m in range(MC)
                ]
                if j < JC - 1:
                    # k-outer/m-inner; all 8 banks accumulate in parallel.
                    for kc in range(KC):
                        for mc in range(MC):
                            mm(ps[mc], mc, kc, j, kc == 0, kc == KC - 1)
                    # pair drains: DVE even banks / ACT odd banks; one DMA
                    # per pair, alternating rings.
                    for mp in range(MC // 2):
                        osb = opool.tile(
                            [128, 1024], F16, name=f"osb{mp}", tag=f"osb{mp}"
                        )
                        nc.vector.tensor_copy(osb[:, :512], ps[2 * mp][:])
                        nc.scalar.copy(osb[:, 512:], ps[2 * mp + 1][:])
                        out_dma(
                            yt16[:, 2 * mp : 2 * mp + 2, j * 512 : (j + 1) * 512],
                            osb[:].rearrange("p (m n) -> p m n", n=512),
                            j,
                        )
                else:
                    # j7: m-outer/k-inner with eager per-bank drain+store so
                    # the kernel tail is one small DMA, not 8 banks' worth.
                    for mc in range(MC - 1):
                        for kc in range(KC):
                            mm(ps[mc], mc, kc, j, kc == 0, kc == KC - 1)
                        osb = opool.tile(
                            [128, 512], F16, name=f"osl{mc}", tag=f"osl{mc}"
                        )
                        if mc % 2 == 0:
                            nc.vector.tensor_copy(osb[:], ps[mc][:])
                        else:
                            nc.scalar.copy(osb[:], ps[mc][:])
                        out_dma(yt16[:, mc, j * 512 : (j + 1) * 512], osb[:], j)
                    # final bank: two independent n-group accumulations
                    # (384 cols, then 128) so the first group drains and
                    # ships while the second computes; the very last store
                    # is only 32KB, shortening the drain+flight chain the
                    # kernel end waits on. Drains stay on DVE (Scalar queue
                    # free to fire its trigger the moment the copy lands).
                    mc = MC - 1
                    base = j * 512
                    osb = opool.tile([128, 512], F16, name="osl7", tag="osl7")
                    for n0, nw in ((0, 256), (256, 256)):
                        nsl = slice(n0, n0 + nw)
                        for kc in range(KC):
                            nc.tensor.matmul(
                                ps[mc][:, nsl],
                                w16t[kc][:, mc * 128 : (mc + 1) * 128],
                                x16t[j][:, kc * 512 + n0 : kc * 512 + n0 + nw],
                                start=(kc == 0),
                                stop=(kc == KC - 1),
                            )
                        nc.vector.tensor_copy(osb[:, nsl], ps[mc][:, nsl])
                        eng = nc.scalar if n0 == 0 else nc.sync
                        eng.dma_start(
                            yt16[:, mc, base + n0 : base + n0 + nw], osb[:, nsl]
                        )

    nc.compile()
    _NC_CACHE = nc
    return nc


def _run(x: np.ndarray, cores: np.ndarray, trace: bool = False, trace_cores=None):
    from concourse.bass_utils import run_bass_kernel_spmd

    W = build_w(cores)
    x16 = x.astype(np.float16)
    w16d = np.ascontiguousarray(
        W.astype(np.float32).astype(np.float16).reshape(KC, 128, D).transpose(1, 0, 2)
    )

    in_maps = []
    for ci in range(N_CORES):
        sl = slice(ci * NPC, (ci + 1) * NPC)
        xt16_c = np.ascontiguousarray(x16[sl].T)
        in_maps.append({"xt16": xt16_c, "w16": w16d})

    nc = _build_bass()
    kwargs = {}
    if trace_cores is not None:
        kwargs["trace_cores"] = trace_cores
    res = run_bass_kernel_spmd(
        nc, in_maps, core_ids=list(range(N_CORES)), trace=trace, **kwargs
    )

    y = np.empty((B, D), dtype=np.float32)
    for ci in range(N_CORES):
        # yt16 [128, MC, NPC] -> [NPC, D]
        y[ci * NPC : (ci + 1) * NPC, :] = (
            res.results[ci]["yt16"].astype(np.float32).transpose(2, 1, 0).reshape(NPC, D)
        )
    return y, res


def kernel(x: np.ndarray, cores: np.ndarray) -> np.ndarray:
    y, _ = _run(x, cores, trace=False)
    return y
